# revision 3
# baseline (speedup 1.0000x reference)
"""nn_Cropper v6: two fully separated gpsimd phases via HBM round-trip.

Phase A (per box): dma_gather 2 tap rows -> tt-only v-blend -> V [128,512,c4]
bf16 -> HWDGE write V to scratch DRAM. Pool sees ONLY dma_gathers.
Phase B (per box): HWDGE reload V -> one 240-idx d=2 ap_gather (both h-taps,
4KB table) -> tt h-blend -> out. Pool sees ONLY ap_gathers.
One ucode switch total; HWDGE write(b) precedes reload(b) on the same SP
queue (FIFO per issuing engine) so the DRAM RAW dependency is ordered.
"""
import numpy as np
import ml_dtypes
from contextlib import ExitStack

B, NBOX, C, H, W = 8, 100, 3, 1024, 1024
S = 100
C4 = 4
WIN = 512
ELEM = WIN * 2       # f32 per gather element (4KB)
STEP = 64
NIDX = 2 * 128
NPAD = 64
GIW = NIDX // 16     # 16
AGW = 16             # wrapped agidx cols per box (256 slots, 240 used)
VF = WIN * C4 // 2   # f32 elems per partition of a V tile (1024)

_CACHE = {}


def _box_geometry(boxes_b):
    fb = boxes_b.astype(np.float32)
    x0 = np.floor(fb[:, 0] * np.float32(W))
    y0 = np.floor(fb[:, 1] * np.float32(H))
    w0 = np.maximum(np.floor(fb[:, 2] * np.float32(W)), np.float32(1.0))
    h0 = np.maximum(np.floor(fb[:, 3] * np.float32(H)), np.float32(1.0))
    grid = (np.arange(S, dtype=np.float32) + np.float32(0.5)) / np.float32(S)
    sy = np.clip(grid[None, :] * h0[:, None] - np.float32(0.5),
                 np.float32(0.0), (h0 - np.float32(1.0))[:, None]) + y0[:, None]
    sx = np.clip(grid[None, :] * w0[:, None] - np.float32(0.5),
                 np.float32(0.0), (w0 - np.float32(1.0))[:, None]) + x0[:, None]
    yf = np.floor(sy)
    xf = np.floor(sx)
    wy = (sy - yf).astype(np.float32)
    wx = (sx - xf).astype(np.float32)
    y0i = np.clip(yf, 0, H - 1).astype(np.int64)
    y1i = np.clip(yf + 1, 0, H - 1).astype(np.int64)
    x0i = np.clip(xf, 0, W - 1).astype(np.int64)
    x1i = np.clip(xf + 1, 0, W - 1).astype(np.int64)
    return wy, wx, y0i, y1i, x0i, x1i


def _wrap16(vals_2d, dtype):
    nb, n = vals_2d.shape
    sw = (n + 15) // 16
    w = np.zeros((nb, 16, sw), dtype=dtype)
    idx = np.arange(n)
    w[:, idx % 16, idx // 16] = vals_2d
    w = w.transpose(1, 0, 2).reshape(16, nb * sw)
    return np.tile(w, (8, 1))


def _prep_core(image_b, boxes_b):
    wy, wx, y0i, y1i, x0i, x1i = _box_geometry(boxes_b)

    xb = np.minimum((x0i.min(axis=1) // 32) * 32, W - WIN)
    assert (x1i.max(axis=1) - xb).max() <= WIN - 1
    assert (x0i.min(axis=1) - xb).min() >= 0
    col = xb // 32

    gfull = np.zeros((NBOX, 2, 128), dtype=np.int64)
    gfull[:, 0, :S] = y0i * 32 + col[:, None]
    gfull[:, 1, :S] = y1i * 32 + col[:, None]
    gfull[:, 0, S:] = gfull[:, 0, S - 1:S]
    gfull[:, 1, S:] = -1            # trailing negatives: skipped
    assert gfull.max() <= 32767
    gidx_all = _wrap16(gfull.reshape(NBOX, NIDX).astype(np.int16), np.int16)

    arel = (x0i - xb[:, None]).astype(np.int16)
    assert arel.min() >= 0 and arel.max() <= WIN - 2
    ag = np.zeros((NBOX, 256), dtype=np.int16)
    ag[:, 0:S] = arel                # u0 taps
    ag[:, 112:112 + S] = arel + 1    # u1 taps
    agidx_all = _wrap16(ag, np.int16)

    wyb = np.zeros((128, NBOX), dtype=ml_dtypes.bfloat16)
    wyb[:S] = wy.T.astype(ml_dtypes.bfloat16)

    wxb = np.broadcast_to(
        wx.reshape(1, NBOX * S).astype(ml_dtypes.bfloat16), (128, NBOX * S)
    ).copy()

    imgc4 = np.zeros((H, W, C4), dtype=ml_dtypes.bfloat16)
    imgc4[:, :, :C] = image_b.transpose(1, 2, 0).astype(ml_dtypes.bfloat16)
    img_pad = np.zeros((H * W * 2 + NPAD,), dtype=np.float32)
    img_pad[:H * W * 2] = imgc4.reshape(-1).view(np.float32)

    return {
        "img": img_pad.reshape(1, -1),
        "gidx": gidx_all,
        "agidx": agidx_all,
        "wyb": wyb.view(np.float32),
        "wxb": wxb.view(np.float32),
    }


def _build_program():
    import concourse.bass as bass
    import concourse.tile as tile
    from concourse import bacc, mybir

    bf16 = mybir.dt.bfloat16
    f32 = mybir.dt.float32
    i16 = mybir.dt.int16
    Alu = mybir.AluOpType

    nc = bacc.Bacc("TRN2", target_bir_lowering=False, debug=False,
                   enable_asserts=False, num_devices=8)
    img_d = nc.dram_tensor("img", [1, H * W * 2 + NPAD], f32,
                           kind="ExternalInput")
    gidx_d = nc.dram_tensor("gidx", [128, NBOX * GIW], i16,
                            kind="ExternalInput")
    agidx_d = nc.dram_tensor("agidx", [128, NBOX * AGW], i16,
                             kind="ExternalInput")
    wyb_d = nc.dram_tensor("wyb", [128, NBOX // 2], f32,
                           kind="ExternalInput")
    wxb_d = nc.dram_tensor("wxb", [128, NBOX * S // 2], f32,
                           kind="ExternalInput")
    vbuf_d = nc.dram_tensor("vbuf", [NBOX, 128, VF], f32, kind="Internal")
    out_d = nc.dram_tensor("out", [NBOX, C, S, S], f32, kind="ExternalOutput")

    with tile.TileContext(nc) as tc, ExitStack() as ctx:
        const = ctx.enter_context(tc.tile_pool(name="const", bufs=1))
        gidx_s = const.tile([128, NBOX * GIW], i16)
        nc.sync.dma_start(gidx_s[:], gidx_d.ap())
        agidx_s = const.tile([128, NBOX * AGW], i16)
        nc.sync.dma_start(agidx_s[:], agidx_d.ap())
        wyb_s = const.tile([128, NBOX // 2], f32)
        nc.sync.dma_start(wyb_s[:], wyb_d.ap())
        wyb_bf = wyb_s[:].bitcast(bf16)
        wxb_s = const.tile([128, NBOX * S // 2], f32)
        nc.sync.dma_start(wxb_s[:], wxb_d.ap())
        wxb_bf = wxb_s[:].bitcast(bf16)

        nrow = (H * W * 2 + NPAD - ELEM) // STEP
        in_view = bass.AP(img_d.ap().tensor, 0, [[STEP, nrow], [1, ELEM]])

        gpool = ctx.enter_context(tc.tile_pool(name="g", bufs=6))
        dpool = ctx.enter_context(tc.tile_pool(name="d", bufs=4))
        vpool = ctx.enter_context(tc.tile_pool(name="v", bufs=4))
        vlpool = ctx.enter_context(tc.tile_pool(name="vl", bufs=8))
        hpool = ctx.enter_context(tc.tile_pool(name="hv", bufs=6))
        opool = ctx.enter_context(tc.tile_pool(name="o", bufs=4))

        import os as _os
        _reps = int(_os.environ.get("BASS_CROP_REPS", "1"))

        G, V, VL, Hv = {}, {}, {}, {}
        for _r in range(_reps):
            # ---- Phase A: gathers + v-blend + spill V ----
            for m in range(NBOX + 3):
                if m < NBOX:
                    G[m] = gpool.tile([128, 2, ELEM], f32, tag="G", name="G")
                    nc.gpsimd.dma_gather(
                        out_ap=G[m][:], in_ap=in_view,
                        idxs_ap=gidx_s[:, m * GIW:(m + 1) * GIW],
                        num_idxs=NIDX, num_idxs_reg=NIDX - 28,
                        elem_size=ELEM, elem_step=STEP,
                    )
                a = m - 3
                if 0 <= a < NBOX:
                    Gb = G[a][:].bitcast(bf16).rearrange(
                        "p t (x c) -> p t x c", c=C4)
                    Dt = dpool.tile([128, WIN, C4], bf16, tag="Dt",
                                    name="Dt")
                    nc.vector.tensor_tensor(
                        out=Dt[:], in0=Gb[:, 1], in1=Gb[:, 0],
                        op=Alu.subtract)
                    wsl = wyb_bf[:, a:a + 1]
                    wyap = bass.AP(wsl.tensor, wsl.offset,
                                   [list(wsl.ap[0]), [0, WIN], [0, C4]])
                    Dw = dpool.tile([128, WIN, C4], bf16, tag="Dw",
                                    name="Dw")
                    nc.vector.tensor_tensor(
                        out=Dw[:], in0=Dt[:], in1=wyap, op=Alu.mult)
                    V[a] = vpool.tile([128, VF], f32, tag="V", name="V")
                    Vb = V[a][:].bitcast(bf16).rearrange(
                        "p (x c) -> p x c", c=C4)
                    nc.vector.tensor_tensor(
                        out=Vb[:], in0=Dw[:], in1=Gb[:, 0], op=Alu.add)
                    nc.sync.dma_start(vbuf_d.ap()[a], V[a][:])
            # ---- Phase B: reload V + h-gather + h-blend + out ----
            for m in range(NBOX + 7):
                if m < NBOX:
                    VL[m] = vlpool.tile([128, VF], f32, tag="VL", name="VL")
                    nc.sync.dma_start(VL[m][:], vbuf_d.ap()[m])
                b = m - 6
                if 0 <= b < NBOX:
                    Hv[b] = hpool.tile([128, 240, 2], f32, tag="Hv",
                                       name="Hv")
                    nc.gpsimd.ap_gather(
                        out_ap=Hv[b][:],
                        in_ap=VL[b][:].rearrange("p (x c) -> p x c", c=2),
                        idxs_ap=agidx_s[:, b * AGW:b * AGW + 15],
                        channels=128, num_elems=WIN, d=2,
                        num_idxs=240,
                    )
                cc = m - 7
                if 0 <= cc < NBOX:
                    Hva = Hv[cc][:].bitcast(bf16)     # [128, 240, 4]
                    Hv0 = Hva[:, 0:S]
                    Hv1 = Hva[:, 112:112 + S]
                    Dh = dpool.tile([128, S, C4], bf16, tag="Dh", name="Dh")
                    nc.vector.tensor_tensor(
                        out=Dh[:], in0=Hv1, in1=Hv0, op=Alu.subtract)
                    wslice = wxb_bf[:, cc * S:(cc + 1) * S]
                    wap = bass.AP(wslice.tensor, wslice.offset,
                                  [list(wslice.ap[0]), list(wslice.ap[1]),
                                   [0, C4]])
                    DW = dpool.tile([128, S, C4], bf16, tag="DW", name="DW")
                    nc.vector.tensor_tensor(
                        out=DW[:], in0=Dh[:], in1=wap, op=Alu.mult)
                    o = opool.tile([128, C4, S], f32, tag="o", name="o")
                    oap = bass.AP(o[:].tensor, o[:].offset,
                                  [list(o[:].ap[0]), [1, S], [S, C4]])
                    nc.vector.tensor_tensor(
                        out=oap, in0=DW[:], in1=Hv0, op=Alu.add)
                    dst = out_d.ap()[cc].transpose([1, 0, 2])
                    nc.sync.dma_start(dst, o[:S, 0:C, :])

    nc.compile()
    return nc


def kernel(images: np.ndarray, boxes: np.ndarray) -> np.ndarray:
    images = np.asarray(images, dtype=np.float32)
    boxes = np.asarray(boxes, dtype=np.float32)
    assert images.shape == (B, C, H, W) and boxes.shape == (B, NBOX, 4)

    if "nc" not in _CACHE:
        _CACHE["nc"] = _build_program()
    nc = _CACHE["nc"]

    in_maps = [_prep_core(images[b], boxes[b]) for b in range(B)]

    from concourse.bass_utils import run_bass_kernel_spmd
    res = run_bass_kernel_spmd(nc, in_maps, core_ids=list(range(B)))
    out = np.stack([res.results[b]["out"] for b in range(B)], axis=0)
    return out.reshape(B * NBOX, C, S, S)


# revision 4
# speedup vs baseline: 131.0823x; 131.0823x over previous
"""nn_Cropper v6: two fully separated gpsimd phases via HBM round-trip.

Phase A (per box): dma_gather 2 tap rows -> tt-only v-blend -> V [128,512,c4]
bf16 -> HWDGE write V to scratch DRAM. Pool sees ONLY dma_gathers.
Phase B (per box): HWDGE reload V -> one 240-idx d=2 ap_gather (both h-taps,
4KB table) -> tt h-blend -> out. Pool sees ONLY ap_gathers.
One ucode switch total; HWDGE write(b) precedes reload(b) on the same SP
queue (FIFO per issuing engine) so the DRAM RAW dependency is ordered.
"""
import numpy as np
import ml_dtypes
from contextlib import ExitStack

B, NBOX, C, H, W = 8, 100, 3, 1024, 1024
S = 100
C4 = 4
WIN = 512
ELEM = WIN * 2       # f32 per gather element (4KB)
STEP = 64
NIDX = 2 * 128
NPAD = 64
GIW = NIDX // 16     # 16
AGW = 16             # wrapped agidx cols per box (256 slots, 240 used)
VF = WIN * C4 // 2   # f32 elems per partition of a V tile (1024)

_CACHE = {}


def _box_geometry(boxes_b):
    fb = boxes_b.astype(np.float32)
    x0 = np.floor(fb[:, 0] * np.float32(W))
    y0 = np.floor(fb[:, 1] * np.float32(H))
    w0 = np.maximum(np.floor(fb[:, 2] * np.float32(W)), np.float32(1.0))
    h0 = np.maximum(np.floor(fb[:, 3] * np.float32(H)), np.float32(1.0))
    grid = (np.arange(S, dtype=np.float32) + np.float32(0.5)) / np.float32(S)
    sy = np.clip(grid[None, :] * h0[:, None] - np.float32(0.5),
                 np.float32(0.0), (h0 - np.float32(1.0))[:, None]) + y0[:, None]
    sx = np.clip(grid[None, :] * w0[:, None] - np.float32(0.5),
                 np.float32(0.0), (w0 - np.float32(1.0))[:, None]) + x0[:, None]
    yf = np.floor(sy)
    xf = np.floor(sx)
    wy = (sy - yf).astype(np.float32)
    wx = (sx - xf).astype(np.float32)
    y0i = np.clip(yf, 0, H - 1).astype(np.int64)
    y1i = np.clip(yf + 1, 0, H - 1).astype(np.int64)
    x0i = np.clip(xf, 0, W - 1).astype(np.int64)
    x1i = np.clip(xf + 1, 0, W - 1).astype(np.int64)
    return wy, wx, y0i, y1i, x0i, x1i


def _wrap16(vals_2d, dtype):
    nb, n = vals_2d.shape
    sw = (n + 15) // 16
    w = np.zeros((nb, 16, sw), dtype=dtype)
    idx = np.arange(n)
    w[:, idx % 16, idx // 16] = vals_2d
    w = w.transpose(1, 0, 2).reshape(16, nb * sw)
    return np.tile(w, (8, 1))


def _prep_core(image_b, boxes_b):
    wy, wx, y0i, y1i, x0i, x1i = _box_geometry(boxes_b)

    xb = np.minimum((x0i.min(axis=1) // 32) * 32, W - WIN)
    assert (x1i.max(axis=1) - xb).max() <= WIN - 1
    assert (x0i.min(axis=1) - xb).min() >= 0
    col = xb // 32

    gfull = np.zeros((NBOX, 2, 128), dtype=np.int64)
    gfull[:, 0, :S] = y0i * 32 + col[:, None]
    gfull[:, 1, :S] = y1i * 32 + col[:, None]
    gfull[:, 0, S:] = gfull[:, 0, S - 1:S]
    gfull[:, 1, S:] = -1            # trailing negatives: skipped
    assert gfull.max() <= 32767
    gidx_all = _wrap16(gfull.reshape(NBOX, NIDX).astype(np.int16), np.int16)

    arel = (x0i - xb[:, None]).astype(np.int16)
    assert arel.min() >= 0 and arel.max() <= WIN - 2
    ag = np.zeros((NBOX, 256), dtype=np.int16)
    ag[:, 0:S] = arel                # u0 taps
    ag[:, 112:112 + S] = arel + 1    # u1 taps
    agidx_all = _wrap16(ag, np.int16)

    wyb = np.zeros((128, NBOX), dtype=ml_dtypes.bfloat16)
    wyb[:S] = wy.T.astype(ml_dtypes.bfloat16)

    wxb = np.broadcast_to(
        wx.reshape(1, NBOX * S).astype(ml_dtypes.bfloat16), (128, NBOX * S)
    ).copy()

    imgc4 = np.zeros((H, W, C4), dtype=ml_dtypes.bfloat16)
    imgc4[:, :, :C] = image_b.transpose(1, 2, 0).astype(ml_dtypes.bfloat16)
    img_pad = np.zeros((H * W * 2 + NPAD,), dtype=np.float32)
    img_pad[:H * W * 2] = imgc4.reshape(-1).view(np.float32)

    return {
        "img": img_pad.reshape(1, -1),
        "gidx": gidx_all,
        "agidx": agidx_all,
        "wyb": wyb.view(np.float32),
        "wxb": wxb.view(np.float32),
    }


def _build_program():
    import concourse.bass as bass
    import concourse.tile as tile
    from concourse import bacc, mybir

    bf16 = mybir.dt.bfloat16
    f32 = mybir.dt.float32
    i16 = mybir.dt.int16
    Alu = mybir.AluOpType

    nc = bacc.Bacc("TRN2", target_bir_lowering=False, debug=False,
                   enable_asserts=False, num_devices=8)
    img_d = nc.dram_tensor("img", [1, H * W * 2 + NPAD], f32,
                           kind="ExternalInput")
    gidx_d = nc.dram_tensor("gidx", [128, NBOX * GIW], i16,
                            kind="ExternalInput")
    agidx_d = nc.dram_tensor("agidx", [128, NBOX * AGW], i16,
                             kind="ExternalInput")
    wyb_d = nc.dram_tensor("wyb", [128, NBOX // 2], f32,
                           kind="ExternalInput")
    wxb_d = nc.dram_tensor("wxb", [128, NBOX * S // 2], f32,
                           kind="ExternalInput")
    vbuf_d = nc.dram_tensor("vbuf", [NBOX, 128, VF], f32, kind="Internal")
    out_d = nc.dram_tensor("out", [NBOX, C, S, S], f32, kind="ExternalOutput")

    with tile.TileContext(nc) as tc, ExitStack() as ctx:
        const = ctx.enter_context(tc.tile_pool(name="const", bufs=1))
        gidx_s = const.tile([128, NBOX * GIW], i16)
        nc.sync.dma_start(gidx_s[:], gidx_d.ap())
        agidx_s = const.tile([128, NBOX * AGW], i16)
        nc.sync.dma_start(agidx_s[:], agidx_d.ap())
        wyb_s = const.tile([128, NBOX // 2], f32)
        nc.sync.dma_start(wyb_s[:], wyb_d.ap())
        wyb_bf = wyb_s[:].bitcast(bf16)
        wxb_s = const.tile([128, NBOX * S // 2], f32)
        nc.sync.dma_start(wxb_s[:], wxb_d.ap())
        wxb_bf = wxb_s[:].bitcast(bf16)

        nrow = (H * W * 2 + NPAD - ELEM) // STEP
        in_view = bass.AP(img_d.ap().tensor, 0, [[STEP, nrow], [1, ELEM]])

        gpool = ctx.enter_context(tc.tile_pool(name="g", bufs=6))
        dpool = ctx.enter_context(tc.tile_pool(name="d", bufs=4))
        vpool = ctx.enter_context(tc.tile_pool(name="v", bufs=4))
        vlpool = ctx.enter_context(tc.tile_pool(name="vl", bufs=12))
        hpool = ctx.enter_context(tc.tile_pool(name="hv", bufs=6))
        opool = ctx.enter_context(tc.tile_pool(name="o", bufs=4))

        import os as _os
        _reps = int(_os.environ.get("BASS_CROP_REPS", "1"))

        G, V, VL, Hv = {}, {}, {}, {}
        for _r in range(_reps):
            # ---- Phase A: gathers + v-blend + spill V ----
            for m in range(NBOX + 3):
                if m < NBOX:
                    G[m] = gpool.tile([128, 2, ELEM], f32, tag="G", name="G")
                    nc.gpsimd.dma_gather(
                        out_ap=G[m][:], in_ap=in_view,
                        idxs_ap=gidx_s[:, m * GIW:(m + 1) * GIW],
                        num_idxs=NIDX, num_idxs_reg=NIDX - 28,
                        elem_size=ELEM, elem_step=STEP,
                    )
                a = m - 3
                if 0 <= a < NBOX:
                    Gb = G[a][:].bitcast(bf16).rearrange(
                        "p t (x c) -> p t x c", c=C4)
                    Dt = dpool.tile([128, WIN, C4], bf16, tag="Dt",
                                    name="Dt")
                    nc.vector.tensor_tensor(
                        out=Dt[:], in0=Gb[:, 1], in1=Gb[:, 0],
                        op=Alu.subtract)
                    wsl = wyb_bf[:, a:a + 1]
                    wyap = bass.AP(wsl.tensor, wsl.offset,
                                   [list(wsl.ap[0]), [0, WIN], [0, C4]])
                    Dw = dpool.tile([128, WIN, C4], bf16, tag="Dw",
                                    name="Dw")
                    nc.vector.tensor_tensor(
                        out=Dw[:], in0=Dt[:], in1=wyap, op=Alu.mult)
                    V[a] = vpool.tile([128, VF], f32, tag="V", name="V")
                    Vb = V[a][:].bitcast(bf16).rearrange(
                        "p (x c) -> p x c", c=C4)
                    nc.vector.tensor_tensor(
                        out=Vb[:], in0=Dw[:], in1=Gb[:, 0], op=Alu.add)
                    nc.sync.dma_start(vbuf_d.ap()[a], V[a][:])
            # ---- Phase B: reload V + h-gather + h-blend + out ----
            for m in range(NBOX + 11):
                if m < NBOX:
                    VL[m] = vlpool.tile([128, VF], f32, tag="VL", name="VL")
                    nc.sync.dma_start(VL[m][:], vbuf_d.ap()[m])
                b = m - 9
                if 0 <= b < NBOX:
                    Hv[b] = hpool.tile([128, 240, 2], f32, tag="Hv",
                                       name="Hv")
                    nc.gpsimd.ap_gather(
                        out_ap=Hv[b][:],
                        in_ap=VL[b][:].rearrange("p (x c) -> p x c", c=2),
                        idxs_ap=agidx_s[:, b * AGW:b * AGW + 15],
                        channels=128, num_elems=WIN, d=2,
                        num_idxs=240,
                    )
                cc = m - 10
                if 0 <= cc < NBOX:
                    Hva = Hv[cc][:].bitcast(bf16)     # [128, 240, 4]
                    Hv0 = Hva[:, 0:S]
                    Hv1 = Hva[:, 112:112 + S]
                    Dh = dpool.tile([128, S, C4], bf16, tag="Dh", name="Dh")
                    nc.vector.tensor_tensor(
                        out=Dh[:], in0=Hv1, in1=Hv0, op=Alu.subtract)
                    wslice = wxb_bf[:, cc * S:(cc + 1) * S]
                    wap = bass.AP(wslice.tensor, wslice.offset,
                                  [list(wslice.ap[0]), list(wslice.ap[1]),
                                   [0, C4]])
                    DW = dpool.tile([128, S, C4], bf16, tag="DW", name="DW")
                    nc.vector.tensor_tensor(
                        out=DW[:], in0=Dh[:], in1=wap, op=Alu.mult)
                    o = opool.tile([128, C4, S], f32, tag="o", name="o")
                    oap = bass.AP(o[:].tensor, o[:].offset,
                                  [list(o[:].ap[0]), [1, S], [S, C4]])
                    nc.vector.tensor_tensor(
                        out=oap, in0=DW[:], in1=Hv0, op=Alu.add)
                    dst = out_d.ap()[cc].transpose([1, 0, 2])
                    nc.scalar.dma_start(dst, o[:S, 0:C, :])

    nc.compile()
    return nc


def kernel(images: np.ndarray, boxes: np.ndarray) -> np.ndarray:
    images = np.asarray(images, dtype=np.float32)
    boxes = np.asarray(boxes, dtype=np.float32)
    assert images.shape == (B, C, H, W) and boxes.shape == (B, NBOX, 4)

    if "nc" not in _CACHE:
        _CACHE["nc"] = _build_program()
    nc = _CACHE["nc"]

    in_maps = [_prep_core(images[b], boxes[b]) for b in range(B)]

    from concourse.bass_utils import run_bass_kernel_spmd
    res = run_bass_kernel_spmd(nc, in_maps, core_ids=list(range(B)))
    out = np.stack([res.results[b]["out"] for b in range(B)], axis=0)
    return out.reshape(B * NBOX, C, S, S)
